# revision 31
# baseline (speedup 1.0000x reference)
"""Trainium2 Bass kernel for nn_AttentionLayer (pooling, dim=0 softmax).

Computation (full shapes B=64, T=2048, D=256):
    u = tanh(hs @ W^T + b)            [B,T,D]
    scores = u @ v                    [B,T]
    a = softmax(scores, axis=0)       (over the batch axis!)
    s[b] = a[b] @ hs[b]               [B,D]

Sharding: sequence-parallel over T across 8 cores (T_loc = 256). The
dim=0 softmax couples samples but not time steps, so each core's
softmax is fully local; only the final weighted sum needs a cross-core
reduction, done on the host (8 x 64KB partials).

Per-core row ordering is (c, b, t_lo) with c = t//128, which makes the
score matrix land directly in [t_lo, b] orientation per half after one
PE transpose (no DRAM bounce, no element-transpose DMA).

Per-core device pipeline (fp16 compute, f32 PSUM accumulation):
  1. plain DMA loads of xt (d-major) / xn (natural) fp16 group tiles,
     dispatched just-in-time on the scalar/sync HWDGE + gpsimd SWDGE
  2. PE mm1: z^T[e, r] = W-chunk @ Xt-chunk  (PSUM f32, 3-deep pool)
  3. ACT: u = tanh(z + bias)  PSUM -> SBUF fp16, per-partition bias
  4. DVE: t_ec = u_ec * v_ec (two 4x-mode tensor_scalar ops)
  5. PE: ones^T @ t0 + ones^T @ t1 -> scores row [1, 1024] PSUM
     (deferred one group so the in-order PE queue never waits on DVE)
  6. one DVE copy PSUM->SBUF row; gpsimd SBUF->SBUF scatter DMA lands
     rows b of the group in the per-half stage tile [64, 128] f32
  7. tail per half: PE transpose stage -> scmat [128 t_lo, 64 b] f32,
     softmax over b (free dim), normalized weights written into a
     block-diagonal slab via a stride-65 access pattern
  8. PE step4: 128 matmuls lhsT=slab[c][:, b, :], rhs=xn[c,b], b-split
     across two [64, 256] PSUM accumulators so the first half's output
     copy + DMA overlap the second half's matmuls
  9. host sums the 8 partials.
"""

import numpy as np

B, T, D = 64, 2048, 256
NCORES = 8
T_LOC = T // NCORES          # 256
BT = B * T_LOC               # 16384 rows per core
BTG = 1024                   # rows per pipeline group (8 b x 128 t_lo)
PH = 128                     # partitions
NG = BT // BTG               # 16 groups (8 per c-half)
GPH = NG // 2                # groups per half


def build_program():
    import concourse.bacc as bacc
    import concourse.tile as tile
    from concourse import mybir

    F32 = mybir.dt.float32
    F16 = mybir.dt.float16
    AF = mybir.ActivationFunctionType
    AX = mybir.AxisListType

    nc = bacc.Bacc("TRN2", target_bir_lowering=False, debug=False)

    # Host-prepacked fp16 inputs (see prep_core_inputs below). Row order
    # is r = c*8192 + b*128 + t_lo  (c = t//128, t_lo = t%128).
    xt_d = nc.dram_tensor("xt16", [NG, PH, 16, PH], F16, kind="ExternalInput").ap()
    xn_d = nc.dram_tensor(
        "xn16", [2, GPH, PH, 8, D], F16, kind="ExternalInput"
    ).ap()
    wt_d = nc.dram_tensor("wt16", [PH, 2, 2, PH], F16, kind="ExternalInput").ap()
    bias_d = nc.dram_tensor("bias2", [PH, 2], F32, kind="ExternalInput").ap()
    v_d = nc.dram_tensor("v2", [PH, 2], F16, kind="ExternalInput").ap()
    out = nc.dram_tensor("out", [B, D], F32, kind="ExternalOutput").ap()

    with tile.TileContext(nc) as tc:
        with (
            tc.tile_pool(name="singles", bufs=1) as singles,
            tc.tile_pool(name="xnat", bufs=NG) as xnat_pool,
            tc.tile_pool(name="xt", bufs=NG) as xt_pool,
            tc.tile_pool(name="usb", bufs=6) as u_pool,
            tc.tile_pool(name="scrow", bufs=4) as scrow_pool,
            tc.tile_pool(name="small", bufs=8) as small,
        ):
            # ---- constants (wt first: it gates the first matmul) ----
            wt = singles.tile([PH, 2, 2, PH], F16)
            nc.sync.dma_start(out=wt, in_=wt_d)
            bias_sb = singles.tile([PH, 2], F32)
            nc.gpsimd.dma_start(out=bias_sb, in_=bias_d)
            v16 = singles.tile([PH, 2], F16)
            nc.gpsimd.dma_start(out=v16, in_=v_d)
            identity32 = singles.tile([PH, PH], F32)
            from concourse.masks import make_identity
            make_identity(nc, identity32)

            stages = [
                singles.tile([B, PH], F32, name=f"stage{c}", tag=f"stage{c}")
                for c in (0, 1)
            ]
            scmats = [
                singles.tile([PH, B], F32, name=f"scmat{c}", tag=f"scmat{c}")
                for c in (0, 1)
            ]
            slabs = [
                singles.tile([PH, B, B], F16, name=f"slab{c}", tag=f"slab{c}")
                for c in (0, 1)
            ]
            for c in (0, 1):
                nc.gpsimd.memset(slabs[c], 0.0)
            s_sb = singles.tile([B, D], F32)

            xnat_tiles = []
            xt_tiles = []
            uv_q = []

            with (
                tc.tile_pool(name="ups", bufs=3, space="PSUM") as ups_pool,
                tc.tile_pool(name="scps", bufs=2, space="PSUM") as scps_pool,
            ):
                def scores_row(g, u16):
                    c, gl = g // GPH, g % GPH
                    scrow = scrow_pool.tile(
                        [1, BTG], F32, name="scrow", tag="scrow"
                    )
                    # ec-outer: consecutive matmuls share the v stationary
                    sc_list = [
                        scps_pool.tile([1, 512], F32, name="sc_ps", tag="sc_ps")
                        for _ in range(2)
                    ]
                    for ec in range(2):
                        for half in range(2):
                            sl = slice(half * 512, (half + 1) * 512)
                            nc.tensor.matmul(
                                sc_list[half],
                                v16[:, ec:ec + 1],
                                u16[ec][:, sl],
                                start=(ec == 0),
                                stop=(ec == 1),
                            )
                    for half in range(2):
                        sl = slice(half * 512, (half + 1) * 512)
                        nc.vector.tensor_copy(scrow[0:1, sl], sc_list[half])
                    nc.gpsimd.dma_start(
                        out=stages[c][gl * 8:gl * 8 + 8, :],
                        in_=scrow,
                    )

                def load_xt(g, split=False):
                    xt = xt_pool.tile([PH, 16, PH], F16, name="xt", tag="xt")
                    eng = nc.scalar if g % 2 == 0 else nc.sync
                    if split:
                        # halves land separately so mm1's first four
                        # matmuls start after only half the bytes
                        eng.dma_start(out=xt[:, 0:8, :], in_=xt_d[g, :, 0:8, :])
                        eng.dma_start(out=xt[:, 8:16, :], in_=xt_d[g, :, 8:16, :])
                    else:
                        eng.dma_start(out=xt, in_=xt_d[g])
                    xt_tiles.append(xt)

                # xt dispatched 3 groups ahead so a late DMA never gates
                # mm1; xn streams behind on the gpsimd SWDGE queue
                load_xt(0)
                load_xt(1)
                load_xt(2)
                load_xt(3)
                for g in range(NG):
                    c, gl = g // GPH, g % GPH
                    if g + 4 < NG:
                        load_xt(g + 4)
                    xn = xnat_pool.tile([PH, 8, D], F16, name="xn", tag="xn")
                    nc.gpsimd.dma_start(out=xn, in_=xn_d[c, gl])
                    xnat_tiles.append(xn)
                    xt = xt_tiles[g]

                    # ---- mm1 + tanh ----
                    u16 = []
                    for ec in range(2):
                        u_ps = ups_pool.tile([PH, BTG], F32)
                        for half in range(2):
                            for dc in range(2):
                                m0 = half * 8 + dc
                                nc.tensor.matmul(
                                    u_ps[:, half * 512:(half + 1) * 512],
                                    wt[:, dc, ec, :],
                                    xt[:, m0:m0 + 7:2, :],
                                    start=(dc == 0),
                                    stop=(dc == 1),
                                )
                        u_sb = u_pool.tile([PH, BTG], F16)
                        nc.scalar.activation(
                            u_sb, u_ps, AF.Tanh, bias=bias_sb[:, ec:ec + 1]
                        )
                        u16.append(u_sb)

                    # scores for the PREVIOUS group (keeps the PE queue
                    # from waiting on this group's tanh)
                    uv_q.append((g, u16))
                    if len(uv_q) > 1:
                        scores_row(*uv_q.pop(0))

                while uv_q:
                    scores_row(*uv_q.pop(0))

            # ---- tail: softmax both halves, then the weighted sum in
            # two b-split PSUM accumulators so the first half's output
            # copy + DMA overlap the second half's matmuls ----
            with (
                tc.tile_pool(name="s4ps", bufs=2, space="PSUM") as s4_pool,
                tc.tile_pool(name="tps", bufs=2, space="PSUM") as t_pool,
            ):
                def softmax_half(c):
                    t_ps = t_pool.tile([PH, B], F32, name="t_ps", tag="t_ps")
                    nc.tensor.transpose(
                        t_ps, stages[c], identity32[0:B, 0:B]
                    )
                    nc.vector.tensor_copy(scmats[c], t_ps)
                    nm = small.tile([PH, 1], F32, name="nm", tag="nm")
                    nc.vector.reduce_max(
                        nm, scmats[c], axis=AX.X, negate=True
                    )
                    e_sb = small.tile([PH, B], F32, name="e_sb", tag="e_sb")
                    ssum = small.tile([PH, 1], F32, name="ssum", tag="ssum")
                    nc.scalar.activation(
                        e_sb, scmats[c], AF.Exp, bias=nm, accum_out=ssum
                    )
                    rec = small.tile([PH, 1], F32, name="rec", tag="rec")
                    nc.vector.reciprocal(rec, ssum)
                    slab_flat = slabs[c].rearrange("p j b -> p (j b)")
                    st = B + 1
                    diag = slab_flat[:, 0:(B - 1) * st + 1:st]
                    nc.vector.tensor_scalar_mul(diag, e_sb, rec)

                def s4_run(ps, c, b0, start, stop):
                    bs = list(range(b0, b0 + B // 2))
                    for i, b in enumerate(bs):
                        nc.tensor.matmul(
                            ps,
                            slabs[c][:, b, :],
                            xnat_tiles[c * GPH + b // 8][:, b % 8, :],
                            start=(start and i == 0),
                            stop=(stop and i == len(bs) - 1),
                            skip_group_check=True,
                        )

                bh = B // 2
                # c0's chunks run while the c1 softmax chain resolves:
                # PE order [T-c0, A(c0), T-c1, B(c0), A(c1)+out, B(c1)+out]
                softmax_half(0)
                s_psA = s4_pool.tile([B, D], F32, name="s_psA", tag="s_psA")
                s_psB = s4_pool.tile([B, D], F32, name="s_psB", tag="s_psB")
                s4_run(s_psA, 0, 0, True, False)
                softmax_half(1)
                s4_run(s_psB, 0, bh, True, False)
                s4_run(s_psA, 1, 0, False, True)
                nc.vector.tensor_copy(s_sb[0:bh, :], s_psA[0:bh, :])
                nc.sync.dma_start(out=out[0:bh, :], in_=s_sb[0:bh, :])
                s4_run(s_psB, 1, bh, False, True)
                nc.vector.tensor_copy(s_sb[bh:B, :], s_psB[bh:B, :])
                nc.sync.dma_start(out=out[bh:B, :], in_=s_sb[bh:B, :])

    nc.compile()
    return nc


_prog_cache = {}


def _get_program(b_dim=B):
    if "p" not in _prog_cache:
        _prog_cache["p"] = build_program()
    return _prog_cache["p"]


def prep_core_inputs(shard_f32, w, bias, v):
    """Pack one core's [B, T_LOC, D] f32 shard + weights into device
    layouts. Row order: r = c*8192 + b*128 + t_lo."""
    h16 = np.asarray(shard_f32).reshape(B, T_LOC, D).astype(np.float16)
    hr = h16.reshape(B, 2, PH, D)
    # xn16[c, gl, p=t_lo, i=b%8, d]
    xn16 = np.ascontiguousarray(
        hr.reshape(8, 8, 2, PH, D).transpose(2, 0, 3, 1, 4)
    ).reshape(2, GPH, PH, 8, D)
    # xt16[g, p, m, q] = X[row=(c, gl*8+i, q), dc*128+p], m = 2i+dc
    hx = hr.reshape(8, 8, 2, PH, 2, PH).transpose(2, 0, 5, 1, 4, 3)
    xt16 = np.ascontiguousarray(hx).reshape(NG, PH, 16, PH)
    w16 = w.astype(np.float16)
    wt16 = np.ascontiguousarray(
        w16.reshape(2, PH, 2, PH).transpose(3, 2, 0, 1)
    )
    bias2 = np.ascontiguousarray(bias.reshape(2, PH).T).astype(np.float32)
    v2 = np.ascontiguousarray(v.reshape(2, PH).T).astype(np.float16)
    return {
        "xn16": xn16,
        "xt16": xt16,
        "wt16": wt16,
        "bias2": bias2,
        "v2": v2,
    }


def kernel(hidden_states, W_attention, bias_attention, attention_vector):
    from concourse.bass_utils import run_bass_kernel_spmd

    hs = np.asarray(hidden_states, dtype=np.float32)
    w = np.asarray(W_attention, dtype=np.float32)
    bias = np.asarray(bias_attention, dtype=np.float32)
    v = np.asarray(attention_vector, dtype=np.float32)

    nc = _get_program()

    in_maps = []
    for core in range(NCORES):
        shard = np.ascontiguousarray(
            hs[:, core * T_LOC:(core + 1) * T_LOC, :]
        )
        in_maps.append(prep_core_inputs(shard, w, bias, v))

    res = run_bass_kernel_spmd(nc, in_maps, list(range(NCORES)))
    s = np.zeros((B, D), dtype=np.float32)
    for r in res.results:
        s += r["out"]
    return s
